# revision 35
# baseline (speedup 1.0000x reference)
"""CrossAttention Trainium2 kernel.

Reference computation (B=4, C=64, H=W=64, N=H*W=4096):
    q = query.reshape(B,C,N); s = support.reshape(B,C,N)
    Q = Wq@q + bq; K = Wk@s + bk; V = Wv@s + bv          (per batch)
    attn = softmax(Q^T K / sqrt(C), axis=m)               (N x N per batch)
    out = (attn @ V^T)^T + query                          -> [B,C,H,W]

Sharding: 8 cores = 4 batches x 2 halves of the query pixels (n axis).
Each core: n_chunk = 2048 query pixels of one batch, full K/V of that batch.

Algebraic folds baked in (all exact up to fp rounding):
  - bk drops out of softmax entirely (adds a per-n constant to logits).
  - K projection is folded into Q:  S^T = K^T Q = s^T (Wqk q + bqk) with
    Wqk = Wk^T Wq, bqk = Wk^T bq precomputed on the host.  The scores
    matmul then contracts raw s chunks against Qk.
  - V projection is folded out of the inner loop:  attn @ V^T = Wv @ Z
    where Z[c',n] = sum_m s[c',m] E[m,n] accumulates via a host-
    pre-transposed s^T (ones column appended -> row 64 of Z is the softmax
    denominator).  Wv applies once per n-tile after normalization.
  - bv is folded into the residual on the host: qb = query + bv.
  - 1/sqrt(C) is folded into the exp() activation's scale.
  - bqk enters through a ones row appended to q (augmented weight row).

Layout: scores are computed transposed, S^T[m, n] (partition = m), so the
exp() output feeds the Z matmul directly as the moving operand with no
on-chip transposes anywhere.  No max-subtraction: |logits/8| < ~1.5 for
this data distribution, exp is comfortably in fp32 range.

Engine budget per core (TimelineSim): ACT exp is the bound resource
(~55us of lane-cycles); PE ~56us; PSUM = scores 2x3 banks + Z 2x1 = 8.
"""

import numpy as np
import ml_dtypes

B, C, H, W = 4, 64, 64, 64
N = H * W              # 4096 keys per batch
NCORES = 8
NPC = (B * N) // NCORES  # 2048 query pixels per core
NT = NPC // 512        # 4 n-tiles per core
MCH = N // 128         # 32 m-chunks
GRP = [3] * 10 + [2]   # m-chunks per exp group (3 banks of PSUM per group)

_cache = {}


def _build():
    import concourse.bass as bass
    import concourse.tile as tile
    from concourse import bacc, mybir
    from contextlib import ExitStack

    f32 = mybir.dt.float32
    bf16 = mybir.dt.bfloat16
    ts = bass.ts
    EXP = mybir.ActivationFunctionType.Exp

    nc = bacc.Bacc("TRN2", target_bir_lowering=False, debug=False,
                   num_devices=NCORES)

    qb_d = nc.dram_tensor("qb", [C, NPC], f32, kind="ExternalInput").ap()
    # qw = [ q-with-ones-row | wqkT+bqk-row | wvT ] packed: one DMA delivers
    # everything the first exp group's dependency chain needs
    qw_d = nc.dram_tensor("qw", [C + 1, NPC + 2 * C], bf16,
                          kind="ExternalInput").ap()
    sbf_d = nc.dram_tensor("sbf", [C, N], bf16, kind="ExternalInput").ap()
    stf_d = nc.dram_tensor("stf", [128, MCH * (C + 1)], bf16,
                           kind="ExternalInput").ap()
    out_d = nc.dram_tensor("out", [C, NPC], f32, kind="ExternalOutput").ap()

    with tile.TileContext(nc) as tc, ExitStack() as ctx:
        const = ctx.enter_context(tc.tile_pool(name="const", bufs=1))
        data = ctx.enter_context(tc.tile_pool(name="data", bufs=1))
        spool = ctx.enter_context(tc.tile_pool(name="spsum", bufs=2, space="PSUM"))
        epool = ctx.enter_context(tc.tile_pool(name="epool", bufs=12))
        tailp = ctx.enter_context(tc.tile_pool(name="tailp", bufs=4))

        # warm the ACT exp table while DMAs run (table load ~2.7us)
        warm = const.tile([1, 1], f32, tag="warm")
        nc.vector.memset(warm[:], 0.0)
        warm2 = const.tile([1, 1], f32, tag="warm2")
        nc.scalar.activation(warm2[:], warm[:], EXP, scale=1.0)

        # ---- bulk loads.  The DMA transfer engine is effectively one
        # serial resource, so everything rides the sync queue in exactly
        # dependency-chain order: qw (q + weights) -> first half of s
        # (scores lhsT chunks) -> s^T (Z lhsT) -> rest of s -> residual.
        qw_t = data.tile([C + 1, NPC + 2 * C], bf16, tag="qw")
        qbf_t = qw_t[:, 0:NPC]                    # ones row 64 (host-packed)
        wqk_t = qw_t[:, NPC : NPC + C]            # [65,64] lhsT, row 64 = bqk
        wvt_t = qw_t[0:C, NPC + C : NPC + 2 * C]  # [64,64] lhsT for Wv apply
        sbf_t = data.tile([C, N], bf16, tag="sbf")
        stf_t = data.tile([128, MCH, C + 1], bf16, tag="stf")  # s^T, ones col
        qb_t = data.tile([C, NPC], f32, tag="qb")
        nc.sync.dma_start(qw_t[:], qw_d)
        nc.sync.dma_start(sbf_t[:, 0 : N // 4], sbf_d[:, 0 : N // 4])
        nc.sync.dma_start(stf_t[:, :, :], stf_d)
        nc.sync.dma_start(sbf_t[:, N // 4 : N], sbf_d[:, N // 4 : N])
        nc.sync.dma_start(qb_t[:], qb_d)

        # keep the PE busy while the loads land: the HAM clock gate holds a
        # cold PE at 1.2 GHz until ~3.4us of sustained activity, which would
        # double the cost of every matmul on the startup critical path
        wz = const.tile([C, 256], bf16, tag="wz")
        nc.vector.memset(wz[:], 0.0)
        for i in range(2):
            pw = spool.tile([C, 512], f32, tag="z")
            for r in range(7):
                nc.tensor.matmul(pw[0:16, ts(r % 2, 256)], wz[:, 0:16], wz[:],
                                 start=True, stop=True)

        # ---- attention --------------------------------------------------
        # Qk(t) = Wqk @ q(t) + bqk.  Qk(t+1) is projected right after tile
        # t's groups but BEFORE tile t's normalization chain, so its DVE
        # copy isn't stuck behind that chain in the DVE FIFO.  The Wv-apply
        # matmul for tile t is emitted after tile t+1's groups: it depends
        # on the normalization chain, and placing it between n-tiles would
        # stall the PE FIFO (and thus ACT) on that chain.
        Qk_t = data.tile([C, NPC], bf16, tag="Qk")
        HN = 256  # normalization sub-chunk (halves the chain latency)

        def qk_proj(t):
            # "z" tag: its second slot is free mid-tile, and using it keeps
            # the "scores" double-buffer rotation undisturbed
            ps = spool.tile([C, 512], f32, tag="z")
            nc.tensor.matmul(ps[:], wqk_t, qbf_t[:, ts(t, 512)],
                             start=True, stop=True)
            nc.vector.tensor_copy(Qk_t[:, ts(t, 512)], ps[:])

        def wv_apply(t, zn_t):
            o2 = spool.tile([C, 512], f32, tag="scores")
            nc.tensor.matmul(o2[:], wvt_t, zn_t[:], start=True, stop=True)
            ou = tailp.tile([C, 512], f32, tag="out")
            nc.vector.tensor_add(ou[:], o2[:], qb_t[:, ts(t, 512)])
            nc.sync.dma_start(out_d[:, ts(t, 512)], ou[:])

        zn_list = []
        for t in range(NT):
            if t == 0:
                qk_proj(0)
            zt = spool.tile([C + 1, 512], f32, tag="z")
            mi = 0
            # tile 0 spins the pipeline up with short groups: the first exp
            # fires earlier and PE's dependency chains stay short while the
            # clock ramp is still cold
            grp = [1, 2] + [3] * 9 + [2] if t == 0 else GRP
            for gi, g in enumerate(grp):
                if gi == 5 and t + 1 < NT:
                    # project next tile's Qk mid-stream: its PE matmul and
                    # DVE copy drain long before the boundary needs them
                    qk_proj(t + 1)
                sc = spool.tile([128, 3, 512], f32, tag="scores")
                for j in range(g):
                    nc.tensor.matmul(sc[:, j, :], sbf_t[:, ts(mi + j, 128)],
                                     Qk_t[:, ts(t, 512)], start=True, stop=True)
                et = epool.tile([128, 3, 512], bf16, tag="e")
                nc.scalar.activation(et[:, 0:g, :], sc[:, 0:g, :], EXP,
                                     scale=0.125)
                for j in range(g):
                    nc.tensor.matmul(zt[:], stf_t[:, mi + j, :], et[:, j, :],
                                     start=(mi + j == 0), stop=(mi + j == MCH - 1))
                mi += g
            # normalize on DVE/Pool (off the PE queue), in halves to cut
            # the serial chain latency; both recips emitted before the muls
            # (DVE is FIFO: a mul waiting on the Pool broadcast must not
            # block the second recip)
            zn_t = tailp.tile([C, 512], bf16, tag="zn")
            hn = HN
            NH = 512 // hn
            rbs = []
            for h in range(NH):
                hs = bass.ds(h * hn, hn)
                r_t = tailp.tile([1, HN], f32, tag="r")
                nc.vector.reciprocal(r_t[0:1, 0:hn], zt[C : C + 1, hs])
                rb_t = tailp.tile([C, HN], f32, tag="rb")
                nc.gpsimd.partition_broadcast(rb_t[:, 0:hn], r_t[0:1, 0:hn])
                rbs.append(rb_t)
            for h in range(NH):
                hs = bass.ds(h * hn, hn)
                nc.vector.tensor_mul(zn_t[:, hs], zt[0:C, hs], rbs[h][:, 0:hn])
            zn_list.append(zn_t)
            if t >= 1:
                wv_apply(t - 1, zn_list[t - 1])
        wv_apply(NT - 1, zn_list[NT - 1])

    nc.compile()
    return nc


def _prep_inputs(query, support, Wq, bq, Wk, bk, Wv, bv):
    """Host-side shard + marshal. Returns list of 8 in_maps."""
    bf = ml_dtypes.bfloat16
    q = np.asarray(query, np.float32).reshape(B, C, N)
    s = np.asarray(support, np.float32).reshape(B, C, N)
    Wq = np.asarray(Wq, np.float32); Wk = np.asarray(Wk, np.float32)
    Wv = np.asarray(Wv, np.float32)
    bq = np.asarray(bq, np.float32); bv = np.asarray(bv, np.float32)

    qb_full = q + bv[None, :, None]          # fold bv into the residual
    wqk = Wk.T @ Wq                          # fold Wk into the Q projection
    bqk = Wk.T @ bq

    in_maps = []
    for core in range(NCORES):
        b, half = divmod(core, NCORES // B)
        off = half * NPC
        # packed [ q | wqkT+bqk | wvT ]; row C is 1.0 under q (bqk carrier)
        qw = np.ones((C + 1, NPC + 2 * C), np.float32)
        qw[0:C, 0:NPC] = q[b, :, off : off + NPC]
        qw[0:C, NPC : NPC + C] = wqk.T
        qw[C, NPC : NPC + C] = bqk
        qw[0:C, NPC + C : NPC + 2 * C] = Wv.T
        qw[C, NPC + C :] = 0.0
        # s^T chunks with ones column: stf[p, mi, c] = s[c, mi*128+p]
        st = np.ones((MCH, 128, C + 1), np.float32)
        st[:, :, 0:C] = s[b].T.reshape(MCH, 128, C)
        stf = np.ascontiguousarray(st.swapaxes(0, 1)).reshape(128, MCH * (C + 1))
        in_maps.append({
            "qb": np.ascontiguousarray(qb_full[b, :, off : off + NPC]),
            "qw": qw.astype(bf),
            "sbf": np.ascontiguousarray(s[b]).astype(bf),
            "stf": stf.astype(bf),
        })
    return in_maps


def kernel(**inputs):
    from concourse.bass_utils import run_bass_kernel_spmd

    if "nc" not in _cache:
        _cache["nc"] = _build()
    nc = _cache["nc"]

    in_maps = _prep_inputs(**inputs)
    res = run_bass_kernel_spmd(nc, in_maps, list(range(NCORES)))
    out = np.empty((B, C, N), np.float32)
    for core in range(NCORES):
        b, half = divmod(core, NCORES // B)
        off = half * NPC
        out[b, :, off : off + NPC] = res.results[core]["out"]
    return out.reshape(B, C, H, W)
